# revision 43
# baseline (speedup 1.0000x reference)
"""Trainium2 Bass kernel for a transformer block (dense_transformer).

Reference computation (B=2, N=2048, C=1024, 16 heads, head_dim=64):
    x = x + attn(LN1(x))            # attn WITHOUT output projection; heads
                                    # interleaved by the faithful reshape
    out = x + MLP(LN2(x))           # MLP = relu(x@w1+b1)@w2+b2

Sharding: 8 cores; core c handles batch b=c//4 and heads 4g..4g+3 (g=c%4).
Because the reference reshapes [B,H,N,hd]->[B,N,C] without transposing
heads back, head h's attention output occupies output rows
[128h, 128h+128) of batch b: out[b, 128h+r, 64m+d] = attn_h[16r+m, d].
So a (batch, 4-head) shard produces a contiguous [512, 1024] output slab
and the whole residual+MLP for those rows is local to the core.

Structure (v2, fused attention+MLP, ~397us vs 398us baseline):
  - Phase 1: LN1 + PE transposes -> ln1T, QKV matmuls.  x chunks on the
    GPSIMD SWDGE queue (x(0) issues ~6us earlier; no head-of-line
    blocking behind compute-gated sync instructions); kTz zero-padding
    memsets on GPSIMD (on DVE they delayed chunk 0's stats by ~9us);
    v evac fp8 on DVE.
  - Fused phase, one head_pass per head: [next head's scores units
    (fp8 q/k, zero-padded K=128 stationary -- K=64 or M=64 forms make
    the PE HAM read half-utilization and throttle to 1.2GHz, measured
    280us throttled) with the pending MLP work proportionally
    interleaved] -> out(h) as a compact block -> [second half].
    exp on ACT is the critical path (~147us); the per-head 128-row MLP
    slab (MLP1 N=128, MLP2 quarter-column N=256 units) queues right
    after post(h) and fills the PE between scores units.  ln2T via PE
    transposes (the xbar DMA transpose is serialized by Tile against
    other DMA traffic, ~10-15us extra latency, and stalled the MLP).
    PSUM: scores f32 2x2 banks, attn-out 2 shared banks (4 j-groups
    per bank, start=True only on each bank round's first MM -- the
    bank-wide has_written clear covers the later groups' first
    writes), MLP1/MLP2/LN2-transposes share one 2-buf tag (2 banks).
    w2 streams as col-quarters through a 3-slot rotation, re-streamed
    per slab, with the tail reusing the two still-resident quarters.
  - b2 is folded host-side into xown (y = x2 + ff + b2); LN affine and
    qkv/v biases folded as before.  x2/xk in bf16 (err 4.2e-3 vs the
    2e-2 gate).
"""

import os
import sys
from collections import deque
from contextlib import ExitStack

for _p in ("/opt/trn_rl_repo", "/root/.axon_site/_ro/trn_rl_repo"):
    if os.path.isdir(_p) and _p not in sys.path:
        sys.path.insert(0, _p)

import numpy as np

import concourse.bass as bass
import concourse.tile as tile
from concourse import bacc, mybir
from concourse.bass_utils import run_bass_kernel_spmd
from concourse.masks import make_identity

F32 = mybir.dt.float32
BF16 = mybir.dt.bfloat16
FP8 = mybir.dt.float8e4
AF = mybir.ActivationFunctionType
OP = mybir.AluOpType

P = 128
B, N, C = 2, 2048, 1024
H, HD = 16, 64
H4 = 4 * C
EPS = 1e-5
SCALE = 1.0 / 32.0  # 1/sqrt(C)

NH = 4            # heads per core
NPAIR = 2         # head pairs per core
ROWS = NH * P     # output rows per core (512)
NCHUNK = N // P   # 16 sequence chunks
CCH = C // P      # 8 channel chunks
HKN = H4 // P     # 32 hidden chunks

_TS = bass.ts


def _emit(nc):
    x = nc.dram_tensor("x", (N, C), F32, kind="ExternalInput").ap()
    xown = nc.dram_tensor("xown", (ROWS, C), F32, kind="ExternalInput").ap()
    wq = nc.dram_tensor("wq", (C, NH * HD), BF16, kind="ExternalInput").ap()
    wk = nc.dram_tensor("wk", (C, NH * HD), BF16, kind="ExternalInput").ap()
    wv = nc.dram_tensor("wv", (C, NH * HD), BF16, kind="ExternalInput").ap()
    qb = nc.dram_tensor("qb", (NH * HD,), F32, kind="ExternalInput").ap()
    kb = nc.dram_tensor("kb", (NH * HD,), F32, kind="ExternalInput").ap()
    w1 = nc.dram_tensor("w1", (C, H4), BF16, kind="ExternalInput").ap()
    b1 = nc.dram_tensor("b1", (H4,), F32, kind="ExternalInput").ap()
    w2 = nc.dram_tensor("w2", (H4, C), BF16, kind="ExternalInput").ap()
    y = nc.dram_tensor("y", (ROWS, C), F32, kind="ExternalOutput").ap()

    reps = int(os.environ.get("KERNEL_REPS", "1"))
    with tile.TileContext(nc) as tc:
        for _ in range(reps):
            _body(tc, nc, x, xown, wq, wk, wv, qb, kb, w1, b1, w2, y)
    return nc


def _body(tc, nc, x, xown, wq, wk, wv, qb, kb, w1, b1, w2, y):
    with ExitStack() as ctx:
        singles = ctx.enter_context(tc.tile_pool(name="singles", bufs=1))

        # --- constants -------------------------------------------------
        id_b = singles.tile([P, P], BF16)
        make_identity(nc, id_b[:])
        eps_t = singles.tile([P, 1], F32)
        nc.vector.memset(eps_t[:], EPS)

        qb_sb = singles.tile([P, NPAIR], F32)
        nc.sync.dma_start(qb_sb[:], qb.rearrange("(pr p) -> p pr", p=P))
        kb_sb = singles.tile([P, NPAIR], F32)
        nc.sync.dma_start(kb_sb[:], kb.rearrange("(pr p) -> p pr", p=P))
        b1_sb = singles.tile([P, HKN], F32)
        nc.sync.dma_start(b1_sb[:], b1.rearrange("(k p) -> p k", p=P))

        gate = singles.tile([1, 8], BF16)

        # --- HAM warmup: back-to-back matmuls so the PE clock is at
        # 2.4GHz by the time real matmuls start; they run while the
        # first x chunks stream in.
        with (
            tc.tile_pool(name="warm", bufs=1) as wp,
            tc.tile_pool(name="wpp", bufs=1, space="PSUM") as wpp,
        ):
            wsrc = wp.tile([P, 512], BF16)
            nc.vector.memset(wsrc[:], 0.0)
            # preload the ACT function tables (each first use otherwise
            # pays a ~1.3us ACT_TABLE_LOAD mid-pipeline)
            adum = wp.tile([1, 8], F32)
            nc.scalar.activation(adum[:], wsrc[0:1, 0:8], AF.Sqrt,
                                 scale=1.0)
            wps = wpp.tile([P, 512], F32)
            for _ in range(28):
                nc.tensor.matmul(wps[:], id_b[:], wsrc[:],
                                 start=True, stop=True)

        attn = ctx.enter_context(tc.tile_pool(name="attn", bufs=1))
        qT = attn.tile([P, NPAIR, N], FP8)
        # k stored zero-padded per head (head h's 64 d-rows at its
        # native partition offset, other 64 partitions ZERO) so scores
        # matmuls present a full 128x128 stationary: K=64 matmuls read
        # as half-utilization and the PE HAM throttles to 1.2GHz
        # (measured: 280us throttled with K=64/M=64 forms).  q and the
        # padded k are fp8: the softmax exponent error this adds
        # (~0.013 rms pre-normalization) costs ~0.01 abs on the output.
        kTz = attn.tile([P, NH, N], FP8)
        v_sb = attn.tile([P, NH, NCHUNK, HD + 1], FP8)

        # MLP weights pool (right side of SBUF)
        mw = ctx.enter_context(tc.tile_pool(name="mw", bufs=1, side="right"))
        w1r = w1.rearrange("(k p) hh -> p k hh", p=P)
        w2r = w2.rearrange("(k p) c -> p k c", p=P)
        w1a = mw.tile([P, CCH, H4 // 2], BF16, name="w1a")
        w1b = mw.tile([P, CCH, H4 // 2], BF16, name="w1b")

        # ------------- phase 1+2: LN1+transpose, QKV --------------
        with (
            tc.tile_pool(name="ph1", bufs=2) as ph1,
            tc.tile_pool(name="pp_a", bufs=4, space="PSUM") as pp_a,
        ):
            ln1T = ph1.tile([P, CCH, N], BF16, tag="ln1T", bufs=1)
            wq_sb = ph1.tile([P, CCH, NH * HD], BF16, tag="wq", bufs=1)
            wk_sb = ph1.tile([P, CCH, NH * HD], BF16, tag="wk", bufs=1)
            wv_sb = ph1.tile([P, CCH, NH * HD], BF16, tag="wv", bufs=1)
            nc.vector.memset(v_sb[:, :, :, HD:HD + 1], 1.0)

            def qk_block(nb):
                # q/k for seq block [512*nb, 512*nb+512) of all 4 heads
                for pr in range(NPAIR):
                    for iw, (wsb, bias_sb) in enumerate(
                            ((wq_sb, qb_sb), (wk_sb, kb_sb))):
                        ps = pp_a.tile([P, 512], F32, tag="ps", bufs=3,
                                       name=f"qk{nb}_{pr}_{iw}")
                        for kc in range(CCH):
                            nc.tensor.matmul(
                                ps[:], wsb[:, kc, _TS(pr, P)],
                                ln1T[:, kc, _TS(nb, 512)],
                                start=(kc == 0), stop=(kc == CCH - 1))
                        if iw == 0:
                            nc.scalar.activation(
                                qT[:, pr, _TS(nb, 512)], ps[:],
                                AF.Identity,
                                bias=bias_sb[:, pr:pr + 1], scale=1.0)
                        else:
                            for h2 in range(2):
                                dp = h2 * HD
                                nc.scalar.activation(
                                    kTz[dp:dp + HD, 2 * pr + h2,
                                        _TS(nb, 512)],
                                    ps[dp:dp + HD, :], AF.Identity,
                                    bias=bias_sb[dp:dp + HD,
                                                 pr:pr + 1],
                                    scale=1.0)

            for t in range(NCHUNK):
                x_t = ph1.tile([P, C], F32, tag="xt", bufs=3)
                # x chunks on the GPSIMD (SWDGE) queue: the first chunk
                # issues ~6us earlier than behind the sync queue's
                # startup program.
                nc.gpsimd.dma_start(x_t[:], x[_TS(t, P), :])
                if t == 0:
                    # queued so chunk 0's LN chain starts early; w1a
                    # streams during phase 1 (there is DMA headroom) so
                    # the fused MLP can start right after head 1.
                    nc.sync.dma_start(
                        wq_sb[:], wq.rearrange("(k p) m -> p k m", p=P))
                    nc.sync.dma_start(
                        wk_sb[:], wk.rearrange("(k p) m -> p k m", p=P))
                    nc.sync.dma_start(
                        wv_sb[:], wv.rearrange("(k p) m -> p k m", p=P))
                    nc.sync.dma_start(w1a[:], w1r[:, :, 0:H4 // 2])
                if t == 1:
                    # zero-padding memsets on GPSIMD: on the DVE they
                    # would head-of-line block chunk 0's LN stats by
                    # ~9us at kernel start
                    for h in range(NH):
                        dz = 0 if h % 2 else HD
                        nc.gpsimd.memset(kTz[dz:dz + HD, h, :], 0.0)
                stats = ph1.tile([P, 2, 6], F32, tag="st")
                nc.vector.bn_stats(stats[:, 0, :], x_t[:, 0:512])
                nc.vector.bn_stats(stats[:, 1, :], x_t[:, 512:1024])
                mv = ph1.tile([P, 2], F32, tag="mv")
                nc.vector.bn_aggr(mv[:], stats[:])
                rstd = ph1.tile([P, 1], F32, tag="rs")
                nc.scalar.activation(rstd[:], mv[:, 1:2], AF.Sqrt,
                                     bias=eps_t[:], scale=1.0)
                nc.vector.reciprocal(rstd[:], rstd[:])
                nmr = ph1.tile([P, 1], F32, tag="nm")
                nc.vector.tensor_scalar(
                    out=nmr[:], in0=mv[:, 0:1], scalar1=rstd[:],
                    scalar2=-1.0, op0=OP.mult, op1=OP.mult)
                xn = ph1.tile([P, C], BF16, tag="xn", bufs=2)
                nc.scalar.activation(xn[:], x_t[:], AF.Identity,
                                     bias=nmr[:], scale=rstd[:])
                # transpose xn -> ln1T, 4 chunks per psum bank, one
                # pure-copy evacuation per bank (LN affine is folded
                # into the weights host-side)
                for half in range(2):
                    pt = pp_a.tile([P, 4, P], BF16, tag="pt", bufs=3,
                                   name=f"pt{t}_{half}")
                    for i in range(4):
                        k = half * 4 + i
                        nc.tensor.transpose(pt[:, i, :], xn[:, _TS(k, P)],
                                            id_b[:])
                    dst = ln1T[:, half * 4:half * 4 + 4, _TS(t, P)]
                    if half == 0:
                        nc.vector.tensor_copy(dst, pt[:])
                    else:
                        nc.scalar.activation(dst, pt[:], AF.Identity,
                                             scale=1.0)
                # v for this seq chunk, directly in natural layout:
                # v[n, d] = (ln1T chunk).T @ wv; fp8 evac on DVE so
                # the ACT queue never waits on PE completions.
                v_ps = pp_a.tile([P, NH * HD], F32, tag="vps", bufs=2,
                                 name=f"v{t}")
                for kc in range(CCH):
                    nc.tensor.matmul(
                        v_ps[:], ln1T[:, kc, _TS(t, P)], wv_sb[:, kc, :],
                        start=(kc == 0), stop=(kc == CCH - 1))
                nc.vector.tensor_copy(
                    v_sb[:, :, t, 0:HD],
                    v_ps.rearrange("p (h d) -> p h d", d=HD))
                if t % 4 == 3:
                    qk_block(t // 4)

        # --- gated loads that stream during early attention ---------
        nc.gpsimd.tensor_copy(gate[:], qT[0:1, 1, 2040:2048])
        # touch Exp now so its ACT_TABLE_LOAD runs behind phase-1's
        # last evacuation instead of gating the first real exp
        edum = attn.tile([1, 8], F32)
        nc.scalar.activation(edum[:], gate[0:1, 0:8], AF.Ln, scale=1.0)
        nc.scalar.activation(edum[:], gate[0:1, 0:8], AF.Exp, scale=1.0)
        nc.gpsimd.tensor_copy(w1b[0:1, 0, 0:8], gate[:])
        nc.gpsimd.dma_start(w1b[:], w1r[:, :, H4 // 2:H4])

        # ------------- fused phase: attention + MLP -------------------
        with (
            tc.tile_pool(name="ph3", bufs=2) as ph3,
            tc.tile_pool(name="pp_s", bufs=2, space="PSUM") as pp_s,
            tc.tile_pool(name="pp_o", bufs=2, space="PSUM") as pp_o,
            tc.tile_pool(name="pp_m", bufs=2, space="PSUM") as pp_m,
        ):
            x2 = ph3.tile([P, NH, C], BF16, tag="x2", bufs=1)
            ln2T = ph3.tile([P, CCH, ROWS], BF16, tag="ln2T", bufs=1)
            xkh = []

            def xk_dma(h, chain_gate=False):
                # per-head residual rows, streamed (cast f32->bf16 by
                # the SWDGE dma); two in flight via the 2-buf tag
                t_ = ph3.tile([P, C], BF16, tag="xkh", bufs=1,
                              name=f"xk{h}")
                if chain_gate:
                    nc.gpsimd.tensor_copy(t_[0:1, 0:8], gate[:])
                nc.gpsimd.dma_start(t_[:], xown[_TS(h, P), :])
                xkh.append(t_)

            def w2q_dma(nm, cq, chain_gate=False):
                t_ = mw.tile([P, HKN, 256], BF16, tag="w2q", bufs=3,
                             name=nm)
                if chain_gate:
                    nc.gpsimd.tensor_copy(t_[0:1, 0, 0:8], gate[:])
                nc.gpsimd.dma_start(t_[:], w2r[:, :, _TS(cq, 256)])
                return t_

            def scores_emitters(h):
                """expT n-half tiles + 32 emit-callables, each one
                (mc, nb) scores matmul pair (BF16 psum, 1 bank) + its
                exp evacuation (fp8 out).  nb-major order so the second
                n-half tile is first written only after the consumer of
                the previous head's first half is done (bufs=3)."""
                pr = h // 2
                expTs = [ph3.tile([P, 8, N], FP8, tag="expT",
                                  bufs=2, name=f"expT{h}_{hf}")
                         for hf in range(2)]

                def unit(mc, nb):
                    pss = pp_s.tile([P, 1024], F32, tag="ss", bufs=2,
                                    name=f"pss{h}_{mc}_{nb}")
                    for nb2 in range(2):
                        nc.tensor.matmul(
                            pss[:, _TS(nb2, 512)],
                            kTz[:, h, _TS(mc, P)],
                            qT[:, pr, _TS(nb * 2 + nb2, 512)],
                            start=True, stop=True)
                    nc.scalar.activation(
                        expTs[mc // 8][:, mc % 8, _TS(nb, 1024)],
                        pss[:], AF.Exp, scale=SCALE)

                ems = [lambda a=(mc, nb): unit(*a)
                       for mc in range(NCHUNK)
                       for nb in range(2)]
                # attention row a = 16r + m maps to output row r,
                # column block m (the reference's interleaved head
                # reshape); lhsT slice [:, mc, j, :] is a full 128-col
                # stationary for output rows 0:128 of column block j.
                lhss = [t.rearrange("p c (r m) -> p c m r", m=16)
                        for t in expTs]
                return ems, lhss

            def evac_quad(h, q, ps):
                rden = ph3.tile([P, 4, 1], F32, tag="rden",
                                name=f"rden{h}_{q}")
                nc.vector.reciprocal(rden[:], ps[:, :, HD:HD + 1])
                for j4 in range(4):
                    j = q * 4 + j4
                    nc.vector.tensor_scalar(
                        out=x2[:, h, _TS(j, HD)],
                        in0=ps[:, j4, 0:HD],
                        scalar1=rden[:, j4, :], scalar2=None,
                        op0=OP.mult)

            def make_out_work(h, lhss):
                """16 j-group callables; every 4th ends with the quad's
                DVE evacuation.  Two shared pso banks: 4 j-groups per
                bank, start=True only on the bank round's first MM."""
                state = {}

                def jgroup(j):
                    q = j // 4
                    if j % 4 == 0:
                        state["ps"] = pp_o.tile(
                            [P, 4, HD + 1], F32, tag="oo", bufs=2,
                            name=f"pso{h}_{q}")
                    ps = state["ps"]
                    for mc in range(NCHUNK):
                        nc.tensor.matmul(
                            ps[:, j % 4, :],
                            lhss[mc // 8][:, mc % 8, j, :],
                            v_sb[:, h, mc, :],
                            start=(j % 4 == 0 and mc == 0),
                            stop=(mc == NCHUNK - 1))
                    if j % 4 == 3:
                        evac_quad(h, q, ps)
                return [lambda jj=j: jgroup(jj) for j in range(16)]

            def post(h):
                if h + 2 < NH:
                    xk_dma(h + 2)
                nc.vector.tensor_add(x2[:, h, :], x2[:, h, :],
                                     xkh[h][:])
                stats2 = ph3.tile([P, 2, 6], F32, tag="st2")
                nc.vector.bn_stats(stats2[:, 0, :], x2[:, h, 0:512])
                nc.vector.bn_stats(stats2[:, 1, :], x2[:, h, 512:1024])
                mv2 = ph3.tile([P, 2], F32, tag="mv2")
                nc.vector.bn_aggr(mv2[:], stats2[:])
                # rsqrt as exp(-0.5*ln(var+eps)): Ln and Exp live in
                # the same ACT table set, so no ~2.7us table switch
                # lands in the middle of the exp stream (Sqrt would
                # force two per head)
                lvar = ph3.tile([P, 1], F32, tag="lv2")
                nc.scalar.activation(lvar[:], mv2[:, 1:2], AF.Ln,
                                     bias=eps_t[:], scale=1.0)
                rstd2 = ph3.tile([P, 1], F32, tag="rs2")
                nc.scalar.activation(rstd2[:], lvar[:], AF.Exp,
                                     scale=-0.5)
                xn2 = ph3.tile([P, C], BF16, tag="xn2", bufs=1)
                nc.vector.tensor_scalar(
                    out=xn2[:], in0=x2[:, h, :], scalar1=mv2[:, 0:1],
                    scalar2=rstd2[:], op0=OP.subtract, op1=OP.mult)
                # PE transposes (not the xbar DMA): the MLP1 units wait
                # on ln2T, and the xbar's serialized ~15us latency would
                # stall the in-order PE queue right behind it.  The
                # transpose psum reuses the m1 slot (same 1KB).
                for half in range(2):
                    tp = pp_m.tile([P, 4, P], BF16, tag="mm", bufs=2,
                                   name=f"tp{h}_{half}")
                    for i in range(4):
                        k = half * 4 + i
                        nc.tensor.transpose(tp[:, i, :], xn2[:, _TS(k, P)],
                                            id_b[:])
                    nc.vector.tensor_copy(
                        ln2T[:, half * 4:half * 4 + 4, _TS(h, P)], tp[:])

            def mlp1_unit(h1t, hcol0, r0, rn, hk):
                def u():
                    w1h = w1a if hk < HKN // 2 else w1b
                    ho = hk % (HKN // 2)
                    ps = pp_m.tile([P, rn], F32, tag="mm", bufs=2,
                                   name=f"m1_{r0}_{hk}")
                    for kc in range(CCH):
                        nc.tensor.matmul(
                            ps[:], w1h[:, kc, _TS(ho, P)],
                            ln2T[:, kc, r0:r0 + rn],
                            start=(kc == 0), stop=(kc == CCH - 1))
                    nc.vector.tensor_scalar(
                        out=h1t[:, hk, hcol0:hcol0 + rn], in0=ps[:],
                        scalar1=b1_sb[:, hk:hk + 1], scalar2=0.0,
                        op0=OP.add, op1=OP.max)
                return u

            def mlp2_unit(h1t, hcol0, j, w2t, cq):
                # out rows [128j, 128j+128), col quarter cq
                def u():
                    ps = pp_m.tile([P, 256], F32, tag="mm", bufs=2,
                                   name=f"m2_{j}_{cq}")
                    for hk in range(HKN):
                        nc.tensor.matmul(
                            ps[:], h1t[:, hk, hcol0:hcol0 + P],
                            w2t[:, hk, :],
                            start=(hk == 0), stop=(hk == HKN - 1))
                    y_sb = ph3.tile([P, 256], F32, tag="ysb", bufs=2,
                                    name=f"y{j}_{cq}")
                    nc.vector.tensor_add(y_sb[:], ps[:],
                                         x2[:, j, _TS(cq, 256)])
                    # y out on the GPSIMD queue: the sync queue's xbar
                    # transposes carry long waits that would head-of-line
                    # block these stores (and stall the whole MLP chain
                    # through the y_sb WAR).
                    nc.gpsimd.dma_start(y[_TS(j, P), _TS(cq, 256)],
                                        y_sb[:])
                return u

            def head_pass(h, lhss, ems_next, work):
                """One pipeline stage: [first half of the next head's
                scores units with proportional slices of the pending
                MLP work], then out(h) as one compact block (it needs
                ALL of exp(h); its completion also frees the expT
                slots the second-half ems reuse), then the rest."""
                outw = make_out_work(h, lhss)
                nw0 = len(work)

                def seg(ems_seg, frac0, frac1):
                    todo = int(nw0 * frac1) - int(nw0 * frac0)
                    slots = max(len(ems_seg), 1)
                    done = 0
                    for i in range(slots):
                        if i < len(ems_seg):
                            ems_seg[i]()
                        want = todo * (i + 1) // slots - done
                        for _ in range(min(want, len(work))):
                            work.popleft()[1]()
                            done += 1

                seg(ems_next[:16], 0.0, 0.55)
                for j in range(16):
                    outw[j]()
                seg(ems_next[16:], 0.55, 1.0)
                if not ems_next:
                    while work:
                        work.popleft()[1]()

            work = deque()
            xk_dma(0, chain_gate=True)
            xk_dma(1, chain_gate=True)
            # w2 quarter tiles rotate through 3 slots; each head's MLP2
            # consumes quarters 0..3, re-streamed per slab (DMA has
            # headroom in the fused phase).  The first three prefetch
            # behind the phase-1 gate.
            wQ = {}

            def mk_dma(nm, cq, chain=False):
                def d():
                    wQ[nm] = w2q_dma(f"w2{nm}", cq, chain_gate=chain)
                return d

            h1s = []

            def queue_slab(si):
                # MLP for head-block si (rows 128si:128si+128), queued
                # right after post(si); quarter dmas pipelined so the
                # 3-slot rotation never stalls.
                h1t = ph3.tile([P, HKN, P], BF16, tag="h1", bufs=2,
                               name=f"h1_{si}")
                h1s.append(h1t)
                qnames = [f"{si}_{cq}" for cq in range(4)]
                if si == 0:
                    mk_dma(qnames[0], 0, chain=True)()
                    mk_dma(qnames[1], 1, chain=True)()
                    mk_dma(qnames[2], 2, chain=True)()
                elif si < 3:
                    for cq in range(3):
                        work.append((50, mk_dma(qnames[cq], cq)))
                else:
                    # tail slab: quarters 2/3 still resident from s2
                    qnames[2] = "2_2"
                    qnames[3] = "2_3"
                    work.append((50, mk_dma(qnames[1], 1)))
                for hk in range(HKN):
                    work.append(
                        (900, mlp1_unit(h1t, 0, 128 * si, P, hk)))
                if si < 3:
                    work.append((50, mk_dma(qnames[3], 3)))
                    order = [0, 1, 2, 3]
                else:
                    order = [3, 2]
                for cq in order:
                    work.append(
                        (3600, lambda c=cq, t=h1t, q=qnames: mlp2_unit(
                            t, 0, si, wQ[q[c]], c)()))
                    if si == 3 and cq == 2:
                        work.append((50, mk_dma(qnames[0], 0)))
                if si == 3:
                    work.append(
                        (3600, lambda t=h1t, q=qnames: mlp2_unit(
                            t, 0, 3, wQ[q[1]], 1)()))
                    work.append(
                        (3600, lambda t=h1t, q=qnames: mlp2_unit(
                            t, 0, 3, wQ[q[0]], 0)()))

            E0, L0 = scores_emitters(0)
            for e in E0:
                e()
            E1, L1 = scores_emitters(1)
            head_pass(0, L0, E1, work)
            post(0)
            queue_slab(0)
            E2, L2 = scores_emitters(2)
            head_pass(1, L1, E2, work)
            post(1)
            queue_slab(1)
            E3, L3 = scores_emitters(3)
            head_pass(2, L2, E3, work)
            post(2)
            queue_slab(2)
            head_pass(3, L3, [], work)
            post(3)
            queue_slab(3)
            while work:
                work.popleft()[1]()


_NC_CACHE = {}


def _get_nc():
    key = os.environ.get("KERNEL_REPS", "1")
    if key not in _NC_CACHE:
        nc = bacc.Bacc("TRN2", target_bir_lowering=False, debug=False,
                       num_devices=8)
        _emit(nc)
        nc.compile()
        _NC_CACHE[key] = nc
    return _NC_CACHE[key]


def make_in_maps(x, qkv_w, qkv_b, w1, b1, w2, b2, ln1_g, ln1_b, ln2_g, ln2_b):
    import ml_dtypes
    x = np.asarray(x, dtype=np.float32)
    qkv_w = np.asarray(qkv_w, dtype=np.float32)
    qkv_b = np.asarray(qkv_b, dtype=np.float32)
    w1 = np.asarray(w1, dtype=np.float32)
    b1 = np.asarray(b1, dtype=np.float32)
    w2 = np.asarray(w2, dtype=np.float32)
    b2 = np.asarray(b2, dtype=np.float32)
    g1 = np.asarray(ln1_g, np.float32)
    bb1 = np.asarray(ln1_b, np.float32)
    g2 = np.asarray(ln2_g, np.float32)
    bb2 = np.asarray(ln2_b, np.float32)

    # Fold LN affine transforms into the downstream weights:
    #   qkv(LN1(x)) = (core1(x) * g1 + bb1) @ W + b
    #               = core1(x) @ (g1[:,None]*W) + (bb1 @ W + b)
    # and likewise LN2 into w1/b1.  The kernel then computes only the
    # core (x-mu)*rstd normalization on-chip.
    qkv_w_eff = g1[:, None] * qkv_w
    qkv_b_eff = qkv_b + bb1 @ qkv_w
    w1_eff = np.ascontiguousarray(
        (g2[:, None] * w1).astype(ml_dtypes.bfloat16))
    b1_eff = b1 + bb2 @ w1
    w2_bf = np.ascontiguousarray(w2.astype(ml_dtypes.bfloat16))

    vb_full = qkv_b_eff[2 * C:]
    in_maps = []
    for core in range(8):
        b, g = divmod(core, 4)
        cs = slice(256 * g, 256 * (g + 1))
        # Fold the v-bias into the residual input: softmax rows sum to 1,
        # so attention(v + 1*vb) = attention(v) + vb broadcast over rows.
        # In the interleaved output layout head h's vb tiles 16x along
        # the channels of its 128-row block.  b2 is also folded here
        # (y = x2 + ff + b2).
        xown = x[b, 512 * g:512 * (g + 1)].copy()
        xown += b2[None, :]
        vb_core = vb_full[cs]
        for hl in range(NH):
            pat = np.tile(vb_core[64 * hl:64 * (hl + 1)], 16)
            xown[128 * hl:128 * (hl + 1), :] += pat[None, :]
        in_maps.append({
            "x": np.ascontiguousarray(x[b]),
            "xown": np.ascontiguousarray(xown),
            "wq": np.ascontiguousarray(
                qkv_w_eff[:, cs].astype(ml_dtypes.bfloat16)),
            "wk": np.ascontiguousarray(
                qkv_w_eff[:, C:2 * C][:, cs].astype(ml_dtypes.bfloat16)),
            "wv": np.ascontiguousarray(
                qkv_w_eff[:, 2 * C:][:, cs].astype(ml_dtypes.bfloat16)),
            "qb": np.ascontiguousarray(qkv_b_eff[cs]),
            "kb": np.ascontiguousarray(qkv_b_eff[C:2 * C][cs]),
            "w1": w1_eff, "b1": b1_eff, "w2": w2_bf,
        })
    return in_maps


def kernel(x, qkv_w, qkv_b, w1, b1, w2, b2, ln1_g, ln1_b, ln2_g, ln2_b):
    nc = _get_nc()
    in_maps = make_in_maps(x, qkv_w, qkv_b, w1, b1, w2, b2,
                           ln1_g, ln1_b, ln2_g, ln2_b)
    res = run_bass_kernel_spmd(nc, in_maps, core_ids=list(range(8)))
    out = np.empty((B, N, C), dtype=np.float32)
    for core in range(8):
        b, g = divmod(core, 4)
        out[b, 512 * g:512 * (g + 1)] = res.results[core]["y"]
    return out
